# revision 16
# baseline (speedup 1.0000x reference)
"""ChebConv + multi-head GAT fused Trainium2 kernel.

Sharding: data-parallel over batch B=8 -> one NeuronCore per batch element.
Each core runs the full per-(b,t) pipeline for its 12 timesteps:

  X_t [N,C] --theta_k--> Z [K*N, F]  --stacked-chebW--> chebT [F,N] (relu)
  chebT --Wq/Wk (head-padded)--> qT,kT [128,N]  --> scores^T per head (PE row-tiled)
  exp on ACT (no max-subtraction; scores are small) -> E^T in SBUF
  attnV: lhsT=[V_h | ones] col-tiled per head -> O2 psum rows = [O_h^T ; s_h replicated]
  normalize via reciprocal + one tensor_tensor (partition-shifted divisor)
  out_proj + fc on PE with per-partition bias folded into DVE PSUM->SBUF copies.

All layout transposes are done host-side in numpy (same bytes moved, DMA-friendly).
"""

from contextlib import ExitStack

import numpy as np

import concourse.bass as bass
import concourse.mybir as mybir
import concourse.tile as tile
from concourse.bass_utils import run_bass_kernel_spmd

F32 = mybir.dt.float32
AF = mybir.ActivationFunctionType
OP = mybir.AluOpType

B, N, C, T = 8, 512, 64, 12
K, F, H = 3, 64, 4
D = F // H  # 16
NC_ = 4     # n-chunks of 128
TPAIRS = T // 2


class Ctx:
    pass


def _load_consts(nc, cx):
    names = ["thetar", "wq", "wk", "bq", "bk", "wv", "wo", "bo", "wf", "bf",
             "shm"]
    shapes = {"thetar": [C, K * F], "wq": [F, 128], "wk": [F, 128],
              "bq": [128, 1], "bk": [128, 1], "wv": [F, F],
              "wo": [112, F], "bo": [F, 1], "wf": [F, F], "bf": [F, 1],
              "shm": [128, 112]}
    for nm in names:
        d = nc.dram_tensor(nm, shapes[nm], F32, kind="ExternalInput")
        t = cx.consts.tile(shapes[nm], F32, tag=nm)
        nc.sync.dma_start(out=t, in_=d[:, :])
        setattr(cx, nm, t)
    bv_d = nc.dram_tensor("bv", [1, F], F32, kind="ExternalInput")
    cx.vbias = cx.consts.tile([128, F], F32, tag="vbias", name="vbias")
    nc.sync.dma_start(out=cx.vbias, in_=bv_d[:, :].to_broadcast((128, F)))
    chebw_d = nc.dram_tensor("chebw", [K * N, N], F32, kind="ExternalInput")
    cx.chebw = []
    for q in range(12):
        cw = cx.chebw_pool.tile([128, N], F32, tag=f"cw{q}", name="cw")
        nc.sync.dma_start(out=cw, in_=chebw_d[q * 128:(q + 1) * 128, :])
        cx.chebw.append(cw)


def _stage_cheb(nc, cx, p):
    """Z matmuls + stacked cheb matmul for timestep pair p -> chebT[0], chebT[1]."""
    xt_sb = []
    for ti in range(2):
        xt = cx.xt_pool.tile([C, N], F32, tag="xt", name="xt")
        nc.sync.dma_start(out=xt, in_=cx.xt_d[2 * p + ti, :, :])
        xt_sb.append(xt)
    z_sb = [cx.z_pool.tile([128, K, 2, F], F32, tag="z", name="z") for _ in range(NC_)]
    for ti in range(2):
        for ic in range(NC_):
            zp = cx.misc_psum.tile([128, K * F], F32, tag="mp", name="zp")
            nc.tensor.matmul(zp, xt_sb[ti][:, ic * 128:(ic + 1) * 128],
                             cx.thetar, start=True, stop=True)
            nc.vector.tensor_copy(z_sb[ic][:, :, ti, :],
                                  zp.rearrange("p (k f) -> p k f", f=F))
    chebp = cx.chebp_psum.tile([128, N], F32, tag="chebp", name="chebp")
    for q in range(12):
        nc.tensor.matmul(chebp, z_sb[q % 4][:, q // 4, :, :], cx.chebw[q],
                         start=(q == 0), stop=(q == 11))
    chebT = []
    for ti in range(2):
        cb = cx.chebT_pool.tile([F, N], F32, tag="chebT", name="chebT")
        if ti == 0:
            nc.vector.tensor_scalar(cb, chebp[0:F, :], 0.0, None, OP.max)
        else:
            # nonzero partition base limits PSUM access to one 32-quad
            nc.vector.tensor_scalar(cb[0:32, :], chebp[64:96, :], 0.0, None, OP.max)
            nc.vector.tensor_scalar(cb[32:64, :], chebp[96:128, :], 0.0, None, OP.max)
        chebT.append(cb)
    return chebT


def _stage_qkv(nc, cx, cb):
    qtp = cx.misc_psum.tile([128, N], F32, tag="mp", name="qtp")
    nc.tensor.matmul(qtp, cx.wq, cb, start=True, stop=True)
    qT = cx.qk_pool.tile([128, N], F32, tag="qT", name="qT")
    nc.vector.tensor_scalar(qT, qtp, cx.bq[:, 0:1], None, OP.add)

    ktp = cx.misc_psum.tile([128, N], F32, tag="mp", name="ktp")
    nc.tensor.matmul(ktp, cx.wk, cb, start=True, stop=True)
    kT = cx.qk_pool.tile([128, N], F32, tag="kT", name="kT")
    nc.vector.tensor_scalar(kT, ktp, cx.bk[:, 0:1], None, OP.add)

    # V'' per (mc, h): 32 cols = [V_h (16) | ones (16)] -> attnV writes
    # [O_h^T ; s_h replicated x16] into a 32-row block. One tile per mc keeps
    # the per-matmul wait fan-in low.
    v_sb = []
    for mc in range(NC_):
        vm = cx.v_pool.tile([128, H, 32], F32, tag=f"v{mc}", name="vm")
        vp = cx.misc_psum.tile([128, F], F32, tag="mp", name="vp")
        nc.tensor.matmul(vp, cb[:, mc * 128:(mc + 1) * 128], cx.wv,
                         start=True, stop=True)
        nc.vector.tensor_tensor(vm[:, :, 0:D],
                                vp.rearrange("p (h d) -> p h d", d=D),
                                cx.vbias.rearrange("p (h d) -> p h d", d=D),
                                OP.add)
        nc.vector.memset(vm[:, :, D:2 * D], 1.0)
        v_sb.append(vm)
    return qT, kT, v_sb


def _stage_attn_head(nc, cx, qT, kT, v_sb, o2p, h):
    e_sb = cx.e_pool.tile([128, NC_, N], F32, tag="e", name="e")
    for half in range(2):
        sp = cx.scores_psum.tile([128, 2, N], F32, tag="sp", name="sp")
        for m2 in range(2):
            mc = half * 2 + m2
            nc.tensor.matmul(sp[:, m2, :],
                             kT[32 * h:32 * h + D, mc * 128:(mc + 1) * 128],
                             qT[32 * h:32 * h + D, :],
                             start=True, stop=True, tile_position=(32 * h, 0))
        nc.scalar.activation(e_sb[:, 2 * half:2 * half + 2, :], sp,
                             AF.Exp, scale=0.25)
    for mc in range(NC_):
        nc.tensor.matmul(o2p[32 * h:32 * h + 32, :], v_sb[mc][:, h, :],
                         e_sb[:, mc, :], start=(mc == 0), stop=(mc == 3),
                         tile_position=(0, 32 * h))


def _stage_out(nc, cx, o2p, t):
    # o2p rows per head h: 32h..32h+15 = O_h^T, 32h+16..+31 = s_h (replicated).
    # onT[r] = o2p[r] / o2p[r+16]; valid rows {32h..32h+15}, rest killed by wo
    # zeros. Engines cannot read at nonzero partition bases wider than a quad,
    # so the +16 shift is a PE matmul with a shift permutation matrix.
    o2s = cx.norm_pool.tile([128, N], F32, tag="o2s", name="o2s")
    nc.vector.tensor_copy(o2s, o2p)
    sdp = cx.misc_psum.tile([112, N], F32, tag="mp", name="sdp")
    nc.tensor.matmul(sdp, cx.shm, o2s, start=True, stop=True)
    rs = cx.norm_pool.tile([112, N], F32, tag="rs", name="rs")
    nc.vector.reciprocal(rs, sdp)
    onT = cx.onT_pool.tile([112, N], F32, tag="onT", name="onT")
    nc.vector.tensor_tensor(onT, o2s[0:112, :], rs, OP.mult)

    aop = cx.misc_psum.tile([F, N], F32, tag="mp", name="aop")
    nc.tensor.matmul(aop, cx.wo, onT, start=True, stop=True)
    aoT = cx.aoT_pool.tile([F, N], F32, tag="aoT", name="aoT")
    nc.vector.tensor_scalar(aoT, aop, cx.bo[:, 0:1], None, OP.add)

    fcp = cx.misc_psum.tile([F, N], F32, tag="mp", name="fcp")
    nc.tensor.matmul(fcp, cx.wf, aoT, start=True, stop=True)
    outT = cx.outT_pool.tile([F, N], F32, tag="outT", name="outT")
    nc.vector.tensor_scalar(outT, fcp, cx.bf[:, 0:1], 0.0, OP.add, OP.max)
    nc.sync.dma_start(out=cx.y_d[t, :, :], in_=outT)


def _build_body(nc, cx):
    _load_consts(nc, cx)
    for p in range(TPAIRS):
        chebT = _stage_cheb(nc, cx, p)
        for ti in range(2):
            t = 2 * p + ti
            qT, kT, v_sb = _stage_qkv(nc, cx, chebT[ti])
            o2p = cx.o2p_psum.tile([128, N], F32, tag="o2p", name="o2p")
            for h in range(H):
                _stage_attn_head(nc, cx, qT, kT, v_sb, o2p, h)
            _stage_out(nc, cx, o2p, t)


def _split_matmul_waits(nc):
    """Walrus allows one sync-wait per Matmult (S3_LW); Tile can emit more.
    Move excess waits onto PE NoOps inserted directly before the matmul."""
    import bass_rust
    for fn in nc.m.functions:
        for bb in fn.blocks:
            out = []
            k = 0
            for i in bb.instructions:
                si = i.sync_info
                if (i.opcode != "EventSemaphore"
                        and si is not None and len(si.on_wait) > 1):
                    waits = list(si.on_wait)
                    for w in waits[:-1]:
                        k += 1
                        out.append(bass_rust.InstNoOp(
                            name=f"I-mmw-{bb.name}-{k}", engine=i.engine,
                            sync_info=bass_rust.SyncInfo(on_wait=[w],
                                                         on_update=[])))
                    i.sync_info = bass_rust.SyncInfo(
                        on_wait=[waits[-1]], on_update=list(si.on_update))
                out.append(i)
            bb.instructions = out


def _build_nc():
    nc = bass.Bass()
    cx = Ctx()
    cx.xt_d = nc.dram_tensor("xt", [T, C, N], F32, kind="ExternalInput")
    cx.y_d = nc.dram_tensor("y", [T, F, N], F32, kind="ExternalOutput")
    with tile.TileContext(nc) as tc, ExitStack() as es:
        pools = [
            ("consts", 1, None), ("chebw_pool", 1, None), ("xt_pool", 4, None),
            ("z_pool", 8, None), ("chebT_pool", 4, None), ("qk_pool", 4, None),
            ("v_pool", 2, None), ("e_pool", 3, None), ("norm_pool", 2, None),
            ("onT_pool", 2, None), ("aoT_pool", 2, None), ("outT_pool", 3, None),
            ("scores_psum", 2, "PSUM"), ("o2p_psum", 1, "PSUM"),
            ("chebp_psum", 1, "PSUM"), ("misc_psum", 2, "PSUM"),
        ]
        for nm, bufs, space in pools:
            kw = {"name": nm, "bufs": bufs}
            if space:
                kw["space"] = space
            setattr(cx, nm, es.enter_context(tc.tile_pool(**kw)))
        _build_body(nc, cx)
    _split_matmul_waits(nc)
    return nc


_NC_CACHE = None


def _prep_shared(cheb_poly, theta, in_proj_w, in_proj_b,
                 out_proj_w, out_proj_b, fc_w, fc_b):
    f32 = np.float32
    chebw = np.ascontiguousarray(cheb_poly.reshape(K * N, N), dtype=f32)
    thetar = np.ascontiguousarray(theta.transpose(1, 0, 2).reshape(C, K * F), dtype=f32)

    wq = np.zeros((F, 128), f32)
    wk = np.zeros((F, 128), f32)
    bq = np.zeros((128, 1), f32)
    bk = np.zeros((128, 1), f32)
    for h in range(H):
        sl_pad = slice(32 * h, 32 * h + D)
        sl = slice(h * D, (h + 1) * D)
        wq[:, sl_pad] = in_proj_w[sl, :].T
        wk[:, sl_pad] = in_proj_w[F + h * D:F + (h + 1) * D, :].T
        bq[sl_pad, 0] = in_proj_b[sl]
        bk[sl_pad, 0] = in_proj_b[F + h * D:F + (h + 1) * D]
    wv = np.ascontiguousarray(in_proj_w[2 * F:3 * F, :].T, dtype=f32)   # [f, o]
    bv = np.ascontiguousarray(in_proj_b[2 * F:3 * F].reshape(1, F), dtype=f32)
    wo = np.zeros((112, F), f32)
    for h in range(H):
        wo[32 * h:32 * h + D, :] = out_proj_w[:, h * D:(h + 1) * D].T
    bo = np.ascontiguousarray(out_proj_b.reshape(F, 1), dtype=f32)
    shm = np.zeros((128, 112), f32)
    for r in range(16, 128):
        shm[r, r - 16] = 1.0
    wf = np.ascontiguousarray(fc_w.T, dtype=f32)
    bf = np.ascontiguousarray(fc_b.reshape(F, 1), dtype=f32)
    return dict(chebw=chebw, thetar=thetar, wq=wq, wk=wk, bq=bq, bk=bk,
                wv=wv, bv=bv, wo=wo, bo=bo, wf=wf, bf=bf, shm=shm)


def kernel(x, spatial_attention, cheb_poly, theta, in_proj_w, in_proj_b,
           out_proj_w, out_proj_b, fc_w, fc_b, _want_results=False):
    global _NC_CACHE
    x = np.asarray(x, dtype=np.float32)
    shared = _prep_shared(
        np.asarray(cheb_poly, np.float32), np.asarray(theta, np.float32),
        np.asarray(in_proj_w, np.float32), np.asarray(in_proj_b, np.float32),
        np.asarray(out_proj_w, np.float32), np.asarray(out_proj_b, np.float32),
        np.asarray(fc_w, np.float32), np.asarray(fc_b, np.float32))

    if _NC_CACHE is None:
        _NC_CACHE = _build_nc()
    nc = _NC_CACHE

    in_maps = []
    for b in range(B):
        xt = np.ascontiguousarray(x[b].transpose(2, 1, 0))  # [T, C, N]
        in_maps.append({"xt": xt, **shared})

    res = run_bass_kernel_spmd(nc, in_maps, core_ids=list(range(B)))
    y = np.stack([res.results[b]["y"].transpose(2, 1, 0) for b in range(B)])
    y = np.ascontiguousarray(y, dtype=np.float32)
    if _want_results:
        return y, res
    return y


# revision 22
# speedup vs baseline: 1.6880x; 1.6880x over previous
"""ChebConv + multi-head GAT fused Trainium2 kernel.

Sharding: data-parallel over batch B=8 -> one NeuronCore per batch element.
Each core runs the full per-(b,t) pipeline for its 12 timesteps:

  X_t [N,C] --theta_k--> Z [K*N, F]  --stacked-chebW--> chebT [F,N] (relu)
  chebT --Wq/Wk (head-padded)--> qT,kT [128,N]  --> scores^T per head (PE row-tiled)
  exp on ACT (no max-subtraction; scores are small) -> E^T in SBUF
  attnV: lhsT=[V_h | ones] col-tiled per head -> O2 psum rows = [O_h^T ; s_h replicated]
  normalize via reciprocal + one tensor_tensor (partition-shifted divisor)
  out_proj + fc on PE with per-partition bias folded into DVE PSUM->SBUF copies.

All layout transposes are done host-side in numpy (same bytes moved, DMA-friendly).
"""

from contextlib import ExitStack

import numpy as np

import concourse.bass as bass
import concourse.mybir as mybir
import concourse.tile as tile
from concourse.bass_utils import run_bass_kernel_spmd

F32 = mybir.dt.float32
AF = mybir.ActivationFunctionType
OP = mybir.AluOpType

F32R = mybir.dt.float32r
BF16 = mybir.dt.bfloat16

B, N, C, T = 8, 512, 64, 12
K, F, H = 3, 64, 4
D = F // H  # 16
NC_ = 4     # n-chunks of 128
TPAIRS = T // 2


class Ctx:
    pass


def _load_consts(nc, cx):
    names = ["thetar", "wq", "wk", "bq", "bk", "wv", "wo", "bo", "wf", "bf",
             "shm"]
    shapes = {"thetar": [C, K * F], "wq": [F, 128], "wk": [F, 128],
              "bq": [128, 1], "bk": [128, 1], "wv": [F, 128],
              "wo": [112, F], "bo": [F, 1], "wf": [F, F], "bf": [F, 1],
              "shm": [128, 112]}
    rdts = {"thetar", "wq", "wk", "wv", "wo", "wf"}
    for nm in names:
        dt = F32R if nm in rdts else F32
        d = nc.dram_tensor(nm, shapes[nm], dt, kind="ExternalInput")
        t = cx.consts.tile(shapes[nm], dt, tag=nm)
        nc.sync.dma_start(out=t, in_=d[:, :])
        setattr(cx, nm, t)
    bv_d = nc.dram_tensor("bv", [1, 128], F32, kind="ExternalInput")
    cx.vbias = cx.consts.tile([128, 128], F32, tag="vbias", name="vbias")
    nc.sync.dma_start(out=cx.vbias, in_=bv_d[:, :].to_broadcast((128, 128)))
    chebw_d = nc.dram_tensor("chebw", [K * N, N], F32R, kind="ExternalInput")
    cx.chebw = []
    for q in range(12):
        cw = cx.chebw_pool.tile([128, N], F32R, tag=f"cw{q}", name="cw")
        nc.sync.dma_start(out=cw, in_=chebw_d[q * 128:(q + 1) * 128, :])
        cx.chebw.append(cw)


def _stage_cheb(nc, cx, p):
    """Z matmuls + stacked cheb matmul for timestep pair p -> chebT[0], chebT[1]."""
    xt_sb = []
    for ti in range(2):
        xt = cx.xt_pool.tile([C, N], F32R, tag="xt", name="xt")
        nc.sync.dma_start(out=xt, in_=cx.xt_d[2 * p + ti, :, :])
        xt_sb.append(xt)
    z_sb = [cx.z_pool.tile([128, K, 2, F], F32R, tag="z", name="z")
            for _ in range(NC_)]
    for ti in range(2):
        for ic in range(NC_):
            zp = cx.misc_psum.tile([128, K * F], F32, tag="mp", name="zp")
            nc.tensor.matmul(zp, (xt_sb[ti][:, ic * 128:(ic + 1) * 128]),
                             (cx.thetar), start=True, stop=True)
            nc.vector.tensor_copy(z_sb[ic][:, :, ti, :],
                                  zp.rearrange("p (k f) -> p k f", f=F))
    chebp = cx.chebp_psum.tile([128, N], F32, tag="chebp", name="chebp")
    for q in range(12):
        nc.tensor.matmul(chebp, (z_sb[q % 4][:, q // 4, :, :]), (cx.chebw[q]),
                         start=(q == 0), stop=(q == 11))
    chebT = []
    for ti in range(2):
        cb = cx.chebT_pool.tile([F, N], F32R, tag="chebT", name="chebT")
        if ti == 0:
            nc.vector.tensor_scalar(cb, chebp[0:F, :], 0.0, None, OP.max)
        else:
            # nonzero partition base limits PSUM access to one 32-quad
            nc.vector.tensor_scalar(cb[0:32, :], chebp[64:96, :], 0.0, None, OP.max)
            nc.vector.tensor_scalar(cb[32:64, :], chebp[96:128, :], 0.0, None, OP.max)
        chebT.append(cb)
    return chebT


def _stage_qkv(nc, cx, cb):
    qtp = cx.misc_psum.tile([128, N], F32, tag="mp", name="qtp")
    nc.tensor.matmul(qtp, (cx.wq), (cb), start=True, stop=True)
    qT = cx.qk_pool.tile([128, N], F32R, tag="qT", name="qT")
    nc.vector.tensor_scalar(qT, qtp, cx.bq[:, 0:1], None, OP.add)

    ktp = cx.misc_psum.tile([128, N], F32, tag="mp", name="ktp")
    nc.tensor.matmul(ktp, (cx.wk), (cb), start=True, stop=True)
    kT = cx.qk_pool.tile([128, N], F32R, tag="kT", name="kT")
    nc.vector.tensor_scalar(kT, ktp, cx.bk[:, 0:1], None, OP.add)

    # V'' per (mc, h): 32 cols = [V_h (16) | ones (16)] -> attnV writes
    # [O_h^T ; s_h replicated x16] into a 32-row block. One tile per mc keeps
    # the per-matmul wait fan-in low.
    v_sb = []
    for mc in range(NC_):
        vm = cx.v_pool.tile([128, H, 32], BF16, tag=f"v{mc}", name="vm")
        vp = cx.misc_psum.tile([128, 128], F32, tag="mp", name="vp")
        nc.tensor.matmul(vp, (cb[:, mc * 128:(mc + 1) * 128]), (cx.wv),
                         start=True, stop=True)
        nc.vector.tensor_tensor(vm,
                                vp.rearrange("p (h d) -> p h d", d=2 * D),
                                cx.vbias.rearrange("p (h d) -> p h d", d=2 * D),
                                OP.add)
        v_sb.append(vm)
    return qT, kT, v_sb


def _stage_attn_head(nc, cx, qT, kT, v_sb, o2p, h):
    e_sb = cx.e_pool.tile([128, NC_, N], BF16, tag="e", name="e")
    for half in range(2):
        sp = cx.scores_psum.tile([128, 2, N], F32, tag="sp", name="sp")
        for m2 in range(2):
            mc = half * 2 + m2
            nc.tensor.matmul(sp[:, m2, :],
                             (kT[32 * h:32 * h + D, mc * 128:(mc + 1) * 128]),
                             (qT[32 * h:32 * h + D, :]),
                             start=True, stop=True, tile_position=(32 * h, 0))
        nc.scalar.activation(e_sb[:, 2 * half:2 * half + 2, :], sp,
                             AF.Exp, scale=0.25)
    for mc in range(NC_):
        nc.tensor.matmul(o2p[32 * h:32 * h + 32, :], (v_sb[mc][:, h, :]),
                         (e_sb[:, mc, :]), start=(mc == 0), stop=(mc == 3),
                         tile_position=(0, 32 * h))


def _stage_out(nc, cx, o2p, t):
    # o2p rows per head h: 32h..32h+15 = O_h^T, 32h+16..+31 = s_h (replicated).
    # onT[r] = o2p[r] / o2p[r+16]; valid rows {32h..32h+15}, rest killed by wo
    # zeros. Engines cannot read at nonzero partition bases wider than a quad,
    # so the +16 shift is a PE matmul with a shift permutation matrix.
    o2s = cx.norm_pool.tile([128, N], F32, tag="o2s", name="o2s")
    nc.vector.tensor_copy(o2s, o2p)
    sdp = cx.misc_psum.tile([112, N], F32, tag="mp", name="sdp")
    nc.tensor.matmul(sdp, cx.shm, o2s, start=True, stop=True)
    rs = cx.norm_pool.tile([112, N], F32, tag="rs", name="rs")
    nc.vector.reciprocal(rs, sdp)
    onT = cx.onT_pool.tile([112, N], F32R, tag="onT", name="onT")
    nc.vector.tensor_tensor(onT, o2s[0:112, :], rs, OP.mult)

    aop = cx.misc_psum.tile([F, N], F32, tag="mp", name="aop")
    nc.tensor.matmul(aop, (cx.wo), (onT), start=True, stop=True)
    aoT = cx.aoT_pool.tile([F, N], F32R, tag="aoT", name="aoT")
    nc.vector.tensor_scalar(aoT, aop, cx.bo[:, 0:1], None, OP.add)

    fcp = cx.misc_psum.tile([F, N], F32, tag="mp", name="fcp")
    nc.tensor.matmul(fcp, (cx.wf), (aoT), start=True, stop=True)
    outT = cx.outT_pool.tile([F, N], F32, tag="outT", name="outT")
    nc.vector.tensor_scalar(outT, fcp, cx.bf[:, 0:1], 0.0, OP.add, OP.max)
    nc.sync.dma_start(out=cx.y_d[t, :, :], in_=outT)


def _build_body(nc, cx):
    _load_consts(nc, cx)
    for p in range(TPAIRS):
        chebT = _stage_cheb(nc, cx, p)
        for ti in range(2):
            t = 2 * p + ti
            qT, kT, v_sb = _stage_qkv(nc, cx, chebT[ti])
            o2p = cx.o2p_psum.tile([128, N], F32, tag="o2p", name="o2p")
            for h in range(H):
                _stage_attn_head(nc, cx, qT, kT, v_sb, o2p, h)
            _stage_out(nc, cx, o2p, t)


def _split_matmul_waits(nc):
    """Walrus allows one sync-wait per Matmult (S3_LW); Tile can emit more.
    Move excess waits onto PE NoOps inserted directly before the matmul."""
    import bass_rust
    for fn in nc.m.functions:
        for bb in fn.blocks:
            out = []
            k = 0
            for i in bb.instructions:
                si = i.sync_info
                if (i.opcode != "EventSemaphore"
                        and si is not None and len(si.on_wait) > 1):
                    waits = list(si.on_wait)
                    for w in waits[:-1]:
                        k += 1
                        out.append(bass_rust.InstNoOp(
                            name=f"I-mmw-{bb.name}-{k}", engine=i.engine,
                            sync_info=bass_rust.SyncInfo(on_wait=[w],
                                                         on_update=[])))
                    i.sync_info = bass_rust.SyncInfo(
                        on_wait=[waits[-1]], on_update=list(si.on_update))
                out.append(i)
            bb.instructions = out


def _build_nc():
    nc = bass.Bass()
    cx = Ctx()
    cx.xt_d = nc.dram_tensor("xt", [T, C, N], F32R, kind="ExternalInput")
    cx.y_d = nc.dram_tensor("y", [T, F, N], F32, kind="ExternalOutput")
    with tile.TileContext(nc) as tc, ExitStack() as es:
        pools = [
            ("consts", 1, None), ("chebw_pool", 1, None), ("xt_pool", 4, None),
            ("z_pool", 8, None), ("chebT_pool", 4, None), ("qk_pool", 4, None),
            ("v_pool", 2, None), ("e_pool", 3, None), ("norm_pool", 2, None),
            ("onT_pool", 2, None), ("aoT_pool", 2, None), ("outT_pool", 3, None),
            ("scores_psum", 2, "PSUM"), ("o2p_psum", 1, "PSUM"),
            ("chebp_psum", 1, "PSUM"), ("misc_psum", 2, "PSUM"),
        ]
        for nm, bufs, space in pools:
            kw = {"name": nm, "bufs": bufs}
            if space:
                kw["space"] = space
            setattr(cx, nm, es.enter_context(tc.tile_pool(**kw)))
        _build_body(nc, cx)
    _split_matmul_waits(nc)
    return nc


_NC_CACHE = None


def _prep_shared(cheb_poly, theta, in_proj_w, in_proj_b,
                 out_proj_w, out_proj_b, fc_w, fc_b):
    f32 = np.float32
    chebw = np.ascontiguousarray(cheb_poly.reshape(K * N, N), dtype=f32)
    thetar = np.ascontiguousarray(theta.transpose(1, 0, 2).reshape(C, K * F), dtype=f32)

    wq = np.zeros((F, 128), f32)
    wk = np.zeros((F, 128), f32)
    bq = np.zeros((128, 1), f32)
    bk = np.zeros((128, 1), f32)
    for h in range(H):
        sl_pad = slice(32 * h, 32 * h + D)
        sl = slice(h * D, (h + 1) * D)
        wq[:, sl_pad] = in_proj_w[sl, :].T
        wk[:, sl_pad] = in_proj_w[F + h * D:F + (h + 1) * D, :].T
        bq[sl_pad, 0] = in_proj_b[sl]
        bk[sl_pad, 0] = in_proj_b[F + h * D:F + (h + 1) * D]
    wv = np.zeros((F, 128), f32)   # [f, (h, d'32)]; d'>=16 cols stay zero
    bv = np.ones((1, 128), f32)    # pad cols get bias 1.0 -> ones for s rows
    for h in range(H):
        wv[:, 32 * h:32 * h + D] = in_proj_w[2 * F + h * D:2 * F + (h + 1) * D, :].T
        bv[0, 32 * h:32 * h + D] = in_proj_b[2 * F + h * D:2 * F + (h + 1) * D]
    wo = np.zeros((112, F), f32)
    for h in range(H):
        wo[32 * h:32 * h + D, :] = out_proj_w[:, h * D:(h + 1) * D].T
    bo = np.ascontiguousarray(out_proj_b.reshape(F, 1), dtype=f32)
    shm = np.zeros((128, 112), f32)
    for r in range(16, 128):
        shm[r, r - 16] = 1.0
    wf = np.ascontiguousarray(fc_w.T, dtype=f32)
    bf = np.ascontiguousarray(fc_b.reshape(F, 1), dtype=f32)
    return dict(chebw=chebw, thetar=thetar, wq=wq, wk=wk, bq=bq, bk=bk,
                wv=wv, bv=bv, wo=wo, bo=bo, wf=wf, bf=bf, shm=shm)


def kernel(x, spatial_attention, cheb_poly, theta, in_proj_w, in_proj_b,
           out_proj_w, out_proj_b, fc_w, fc_b, _want_results=False):
    global _NC_CACHE
    x = np.asarray(x, dtype=np.float32)
    shared = _prep_shared(
        np.asarray(cheb_poly, np.float32), np.asarray(theta, np.float32),
        np.asarray(in_proj_w, np.float32), np.asarray(in_proj_b, np.float32),
        np.asarray(out_proj_w, np.float32), np.asarray(out_proj_b, np.float32),
        np.asarray(fc_w, np.float32), np.asarray(fc_b, np.float32))

    if _NC_CACHE is None:
        _NC_CACHE = _build_nc()
    nc = _NC_CACHE

    in_maps = []
    for b in range(B):
        xt = np.ascontiguousarray(x[b].transpose(2, 1, 0))  # [T, C, N]
        in_maps.append({"xt": xt, **shared})

    res = run_bass_kernel_spmd(nc, in_maps, core_ids=list(range(B)))
    y = np.stack([res.results[b]["y"].transpose(2, 1, 0) for b in range(B)])
    y = np.ascontiguousarray(y, dtype=np.float32)
    if _want_results:
        return y, res
    return y


# revision 24
# speedup vs baseline: 1.7549x; 1.0396x over previous
"""ChebConv + multi-head GAT fused Trainium2 kernel.

Sharding: data-parallel over batch B=8 -> one NeuronCore per batch element.
Each core runs the full per-(b,t) pipeline for its 12 timesteps:

  X_t [N,C] --theta_k--> Z [K*N, F]  --stacked-chebW--> chebT [F,N] (relu)
  chebT --Wq/Wk (head-padded)--> qT,kT [128,N]  --> scores^T per head (PE row-tiled)
  exp on ACT (no max-subtraction; scores are small) -> E^T in SBUF
  attnV: lhsT=[V_h | ones] col-tiled per head -> O2 psum rows = [O_h^T ; s_h replicated]
  normalize via reciprocal + one tensor_tensor (partition-shifted divisor)
  out_proj + fc on PE with per-partition bias folded into DVE PSUM->SBUF copies.

All layout transposes are done host-side in numpy (same bytes moved, DMA-friendly).
"""

from contextlib import ExitStack

import numpy as np

import concourse.bass as bass
import concourse.mybir as mybir
import concourse.tile as tile
from concourse.bass_utils import run_bass_kernel_spmd

F32 = mybir.dt.float32
AF = mybir.ActivationFunctionType
OP = mybir.AluOpType

F32R = mybir.dt.float32r
BF16 = mybir.dt.bfloat16

B, N, C, T = 8, 512, 64, 12
K, F, H = 3, 64, 4
D = F // H  # 16
NC_ = 4     # n-chunks of 128
TPAIRS = T // 2


class Ctx:
    pass


def _load_consts(nc, cx):
    names = ["thetar", "wq", "wk", "bq", "bk", "wv", "wo", "bo", "wf", "bf",
             "shm"]
    shapes = {"thetar": [C, 256], "wq": [F, 128], "wk": [F, 128],
              "bq": [128, 1], "bk": [128, 1], "wv": [F, 128],
              "wo": [112, F], "bo": [F, 1], "wf": [F, F], "bf": [F, 1],
              "shm": [128, 112]}

    rdts = {"thetar", "wq", "wk", "wv", "wo", "wf", "shm"}
    for nm in names:
        dt = F32R if nm in rdts else F32
        d = nc.dram_tensor(nm, shapes[nm], dt, kind="ExternalInput")
        t = cx.consts.tile(shapes[nm], dt, tag=nm)
        nc.sync.dma_start(out=t, in_=d[:, :])
        setattr(cx, nm, t)
    bv_d = nc.dram_tensor("bv", [1, 128], F32, kind="ExternalInput")
    cx.vbias = cx.consts.tile([128, 128], F32, tag="vbias", name="vbias")
    nc.sync.dma_start(out=cx.vbias, in_=bv_d[:, :].to_broadcast((128, 128)))
    chebw_d = nc.dram_tensor("chebw", [K * N, N], F32R, kind="ExternalInput")
    cx.chebw = []
    for q in range(12):
        cw = cx.chebw_pool.tile([128, N], F32R, tag=f"cw{q}", name="cw")
        nc.sync.dma_start(out=cw, in_=chebw_d[q * 128:(q + 1) * 128, :])
        cx.chebw.append(cw)


def _stage_cheb(nc, cx, p):
    """Z matmuls + stacked cheb matmul for timestep pair p -> chebT[0], chebT[1]."""
    xt_sb = []
    for ti in range(2):
        xt = cx.xt_pool.tile([C, N], F32R, tag="xt", name="xt")
        nc.sync.dma_start(out=xt, in_=cx.xt_d[2 * p + ti, :, :])
        xt_sb.append(xt)
    z_sb = [cx.z_pool.tile([128, K, 2, F], F32R, tag="z", name="z")
            for _ in range(NC_)]
    for ti in range(2):
        for ic in range(NC_):
            zp = cx.misc_psum.tile([128, 256], F32, tag="mp", name="zp")
            nc.tensor.matmul(zp, (xt_sb[ti][:, ic * 128:(ic + 1) * 128]),
                             (cx.thetar), start=True, stop=True)
            nc.vector.tensor_copy(z_sb[ic][:, :, ti, :],
                                  zp[:, 0:K * F].rearrange("p (k f) -> p k f",
                                                           f=F))
    chebp = cx.chebp_psum.tile([128, N], F32, tag="chebp", name="chebp")
    for q in range(12):
        nc.tensor.matmul(chebp, (z_sb[q % 4][:, q // 4, :, :]), (cx.chebw[q]),
                         start=(q == 0), stop=(q == 11))
    chebT = []
    for ti in range(2):
        cb = cx.chebT_pool.tile([F, N], F32R, tag="chebT", name="chebT")
        if ti == 0:
            nc.vector.tensor_scalar(cb, chebp[0:F, :], 0.0, None, OP.max)
        else:
            # nonzero partition base limits PSUM access to one 32-quad
            nc.vector.tensor_scalar(cb[0:32, :], chebp[64:96, :], 0.0, None, OP.max)
            nc.vector.tensor_scalar(cb[32:64, :], chebp[96:128, :], 0.0, None, OP.max)
        chebT.append(cb)
    return chebT


def _stage_qkv(nc, cx, cb):
    qtp = cx.misc_psum.tile([128, N], F32, tag="mp", name="qtp")
    nc.tensor.matmul(qtp, (cx.wq), (cb), start=True, stop=True)
    qT = cx.qk_pool.tile([128, N], F32R, tag="qT", name="qT")
    nc.vector.tensor_scalar(qT, qtp, cx.bq[:, 0:1], None, OP.add)

    ktp = cx.misc_psum.tile([128, N], F32, tag="mp", name="ktp")
    nc.tensor.matmul(ktp, (cx.wk), (cb), start=True, stop=True)
    kT = cx.qk_pool.tile([128, N], F32R, tag="kT", name="kT")
    nc.vector.tensor_scalar(kT, ktp, cx.bk[:, 0:1], None, OP.add)

    # V'' per (mc, h): 32 cols = [V_h (16) | ones (16)] -> attnV writes
    # [O_h^T ; s_h replicated x16] into a 32-row block. One tile per mc keeps
    # the per-matmul wait fan-in low.
    v_sb = []
    for mc in range(NC_):
        vm = cx.v_pool.tile([128, H, 32], BF16, tag=f"v{mc}", name="vm")
        vp = cx.misc_psum.tile([128, 128], F32, tag="mp", name="vp")
        nc.tensor.matmul(vp, (cb[:, mc * 128:(mc + 1) * 128]), (cx.wv),
                         start=True, stop=True)
        nc.vector.tensor_tensor(vm,
                                vp.rearrange("p (h d) -> p h d", d=2 * D),
                                cx.vbias.rearrange("p (h d) -> p h d", d=2 * D),
                                OP.add)
        v_sb.append(vm)
    return qT, kT, v_sb


def _stage_attn_head(nc, cx, qT, kT, v_sb, o2p, h):
    e_sb = cx.e_pool.tile([128, NC_, N], BF16, tag="e", name="e")
    for half in range(2):
        sp = cx.scores_psum.tile([128, 2, N], F32, tag="sp", name="sp")
        for m2 in range(2):
            mc = half * 2 + m2
            nc.tensor.matmul(sp[:, m2, :],
                             (kT[32 * h:32 * h + D, mc * 128:(mc + 1) * 128]),
                             (qT[32 * h:32 * h + D, :]),
                             start=True, stop=True, tile_position=(32 * h, 0))
        nc.scalar.activation(e_sb[:, 2 * half:2 * half + 2, :], sp,
                             AF.Exp, scale=0.25)
    for mc in range(NC_):
        nc.tensor.matmul(o2p[32 * h:32 * h + 32, :], (v_sb[mc][:, h, :]),
                         (e_sb[:, mc, :]), start=(mc == 0), stop=(mc == 3),
                         tile_position=(0, 32 * h))


def _stage_out(nc, cx, o2p, t):
    # o2p rows per head h: 32h..32h+15 = O_h^T, 32h+16..+31 = s_h (replicated).
    # onT[r] = o2p[r] / o2p[r+16]; valid rows {32h..32h+15}, rest killed by wo
    # zeros. Engines cannot read at nonzero partition bases wider than a quad,
    # so the +16 shift is a PE matmul with a shift permutation matrix.
    o2s = cx.norm_pool.tile([128, N], F32R, tag="o2s", name="o2s")
    nc.vector.tensor_copy(o2s, o2p)
    sdp = cx.misc_psum.tile([112, N], F32, tag="mp", name="sdp")
    nc.tensor.matmul(sdp, cx.shm, o2s, start=True, stop=True)
    rs = cx.norm_pool.tile([112, N], F32, tag="rs", name="rs")
    nc.vector.reciprocal(rs, sdp)
    onT = cx.onT_pool.tile([112, N], F32R, tag="onT", name="onT")
    nc.vector.tensor_tensor(onT, o2s[0:112, :].bitcast(F32), rs, OP.mult)

    aop = cx.misc_psum.tile([F, N], F32, tag="mp", name="aop")
    nc.tensor.matmul(aop, (cx.wo), (onT), start=True, stop=True)
    aoT = cx.aoT_pool.tile([F, N], F32R, tag="aoT", name="aoT")
    nc.vector.tensor_scalar(aoT, aop, cx.bo[:, 0:1], None, OP.add)

    fcp = cx.misc_psum.tile([F, N], F32, tag="mp", name="fcp")
    nc.tensor.matmul(fcp, (cx.wf), (aoT), start=True, stop=True)
    outT = cx.outT_pool.tile([F, N], F32, tag="outT", name="outT")
    nc.vector.tensor_scalar(outT, fcp, cx.bf[:, 0:1], 0.0, OP.add, OP.max)
    nc.sync.dma_start(out=cx.y_d[t, :, :], in_=outT)


def _build_body(nc, cx):
    _load_consts(nc, cx)
    for p in range(TPAIRS):
        chebT = _stage_cheb(nc, cx, p)
        for ti in range(2):
            t = 2 * p + ti
            qT, kT, v_sb = _stage_qkv(nc, cx, chebT[ti])
            o2p = cx.o2p_psum.tile([128, N], F32, tag="o2p", name="o2p")
            for h in range(H):
                _stage_attn_head(nc, cx, qT, kT, v_sb, o2p, h)
            _stage_out(nc, cx, o2p, t)


def _split_matmul_waits(nc):
    """Walrus allows one sync-wait per Matmult (S3_LW); Tile can emit more.
    Move excess waits onto PE NoOps inserted directly before the matmul."""
    import bass_rust
    for fn in nc.m.functions:
        for bb in fn.blocks:
            out = []
            k = 0
            for i in bb.instructions:
                si = i.sync_info
                if (i.opcode != "EventSemaphore"
                        and si is not None and len(si.on_wait) > 1):
                    waits = list(si.on_wait)
                    for w in waits[:-1]:
                        k += 1
                        out.append(bass_rust.InstNoOp(
                            name=f"I-mmw-{bb.name}-{k}", engine=i.engine,
                            sync_info=bass_rust.SyncInfo(on_wait=[w],
                                                         on_update=[])))
                    i.sync_info = bass_rust.SyncInfo(
                        on_wait=[waits[-1]], on_update=list(si.on_update))
                out.append(i)
            bb.instructions = out


def _build_nc():
    nc = bass.Bass()
    cx = Ctx()
    cx.xt_d = nc.dram_tensor("xt", [T, C, N], F32R, kind="ExternalInput")
    cx.y_d = nc.dram_tensor("y", [T, F, N], F32, kind="ExternalOutput")
    with tile.TileContext(nc) as tc, ExitStack() as es:
        pools = [
            ("consts", 1, None), ("chebw_pool", 1, None), ("xt_pool", 4, None),
            ("z_pool", 8, None), ("chebT_pool", 4, None), ("qk_pool", 4, None),
            ("v_pool", 2, None), ("e_pool", 3, None), ("norm_pool", 2, None),
            ("onT_pool", 2, None), ("aoT_pool", 2, None), ("outT_pool", 3, None),
            ("scores_psum", 2, "PSUM"), ("o2p_psum", 1, "PSUM"),
            ("chebp_psum", 1, "PSUM"), ("misc_psum", 2, "PSUM"),
        ]
        for nm, bufs, space in pools:
            kw = {"name": nm, "bufs": bufs}
            if space:
                kw["space"] = space
            setattr(cx, nm, es.enter_context(tc.tile_pool(**kw)))
        _build_body(nc, cx)
    _split_matmul_waits(nc)
    return nc


_NC_CACHE = None


def _prep_shared(cheb_poly, theta, in_proj_w, in_proj_b,
                 out_proj_w, out_proj_b, fc_w, fc_b):
    f32 = np.float32
    chebw = np.ascontiguousarray(cheb_poly.reshape(K * N, N), dtype=f32)
    thetar = np.zeros((C, 256), f32)
    thetar[:, 0:K * F] = theta.transpose(1, 0, 2).reshape(C, K * F)

    wq = np.zeros((F, 128), f32)
    wk = np.zeros((F, 128), f32)
    bq = np.zeros((128, 1), f32)
    bk = np.zeros((128, 1), f32)
    for h in range(H):
        sl_pad = slice(32 * h, 32 * h + D)
        sl = slice(h * D, (h + 1) * D)
        wq[:, sl_pad] = in_proj_w[sl, :].T
        wk[:, sl_pad] = in_proj_w[F + h * D:F + (h + 1) * D, :].T
        bq[sl_pad, 0] = in_proj_b[sl]
        bk[sl_pad, 0] = in_proj_b[F + h * D:F + (h + 1) * D]
    wv = np.zeros((F, 128), f32)   # [f, (h, d'32)]; d'>=16 cols stay zero
    bv = np.ones((1, 128), f32)    # pad cols get bias 1.0 -> ones for s rows
    for h in range(H):
        wv[:, 32 * h:32 * h + D] = in_proj_w[2 * F + h * D:2 * F + (h + 1) * D, :].T
        bv[0, 32 * h:32 * h + D] = in_proj_b[2 * F + h * D:2 * F + (h + 1) * D]
    wo = np.zeros((112, F), f32)
    for h in range(H):
        wo[32 * h:32 * h + D, :] = out_proj_w[:, h * D:(h + 1) * D].T
    bo = np.ascontiguousarray(out_proj_b.reshape(F, 1), dtype=f32)
    shm = np.zeros((128, 112), f32)
    for r in range(16, 128):
        shm[r, r - 16] = 1.0
    wf = np.ascontiguousarray(fc_w.T, dtype=f32)
    bf = np.ascontiguousarray(fc_b.reshape(F, 1), dtype=f32)
    return dict(chebw=chebw, thetar=thetar, wq=wq, wk=wk, bq=bq, bk=bk,
                wv=wv, bv=bv, wo=wo, bo=bo, wf=wf, bf=bf, shm=shm)


def kernel(x, spatial_attention, cheb_poly, theta, in_proj_w, in_proj_b,
           out_proj_w, out_proj_b, fc_w, fc_b, _want_results=False):
    global _NC_CACHE
    x = np.asarray(x, dtype=np.float32)
    shared = _prep_shared(
        np.asarray(cheb_poly, np.float32), np.asarray(theta, np.float32),
        np.asarray(in_proj_w, np.float32), np.asarray(in_proj_b, np.float32),
        np.asarray(out_proj_w, np.float32), np.asarray(out_proj_b, np.float32),
        np.asarray(fc_w, np.float32), np.asarray(fc_b, np.float32))

    if _NC_CACHE is None:
        _NC_CACHE = _build_nc()
    nc = _NC_CACHE

    in_maps = []
    for b in range(B):
        xt = np.ascontiguousarray(x[b].transpose(2, 1, 0))  # [T, C, N]
        in_maps.append({"xt": xt, **shared})

    res = run_bass_kernel_spmd(nc, in_maps, core_ids=list(range(B)))
    y = np.stack([res.results[b]["y"].transpose(2, 1, 0) for b in range(B)])
    y = np.ascontiguousarray(y, dtype=np.float32)
    if _want_results:
        return y, res
    return y
